# revision 59
# baseline (speedup 1.0000x reference)
"""Trainium2 Bass kernel for a multi-head cross-attention module.

Math (validated vs reference to 5.4e-7 in f32):
  Q = x@Wq+bq, K = x@Wk          (N=2048, 8 heads, head_dim=64)
  scores[q,k,h] = <Q[q,h,:], K[k,h,:]>/8
    - spatial bias sb(q): per-query shift along k -> softmax no-op, dropped
    - K bias bk: <Q[q,h],bk[h]> is per-(q,h) shift along k -> softmax
      no-op, dropped (exact)
  A = softmax_k(scores); out[q] = sum_{k,h} A[q,k,h]*U[k,h]/Z[q,h] + bo
  where U[k,h] = mg[k]*(x[k]@Wv_tilde[:,h]+bv_tilde[h]) folds the V
  projection, motion gate and output projection (host-prepped: the
  gate MLP + U are O(N*small), 0.4% of total FLOPs; all O(N*d^2)
  projections and the O(N^2*H) attention run on device).

Sharding: queries split 256/core across 8 cores; K/U replicated.

Per-core dataflow (d = head-pair 0..3 pipelined):
  K-proj (PE, bf16) -> KT staging (ACT/DVE split) ->
  scores S^T[k,q] per key-tile, head pair concurrent on PE row-groups
  (64-row contraction at base partitions 0/64) ->
  exp: even tiles ACT Exp(scale=1/8); odd tiles DVE "Schraudolph"
  (one tensor_scalar producing the bf16 BITS of exp via int16 convert +
  bitcast; end-to-end rel err contribution ~1e-3) ->
  Z/W matmul against [1|U] with 4x PE column-tiling: key-tile kt goes to
  partition strip 32*(kt%4), 4 concurrent streams, one zeroing matmul
  opens the bank -> strips folded by a [128,9] 4-stacked-identity
  matmul (E) which also transposes for the final combine.

Walrus 1-wait constraint handled by _legalize_waits; steady-state the
schedule needs <=1 wait per instruction (vector clocks elide repeats).
"""

import numpy as np
import ml_dtypes
from contextlib import ExitStack

import concourse.bass as bass
import concourse.mybir as mybir
import concourse.tile as tile
from concourse.bass_utils import run_bass_kernel_spmd

N = 2048
CIN = 256
DOUT = 512
H = 8
HD = 64
NCORES = 8
NQ = N // NCORES        # 256 queries per core
NKT = N // 128          # 16 key tiles
F32 = mybir.dt.float32
BF16 = mybir.dt.bfloat16
I16 = mybir.dt.int16
F8 = mybir.dt.float8e4

# Schraudolph: bf16bits(exp(s/8)) ~= int16((s + B) * A)
SC_EXP = 0.125
A_IMM = SC_EXP * 128.0 / float(np.log(2.0))
B_IMM = 16249.0 / A_IMM                    # (127*128 - 7)/A

# engine split per d-iteration (tunable): exp tiles t=0..7, KT chunks f=0..3
EXP_ON_ACT = (True, False, True, False, True, False, True, False)
KT_ON_ACT = (True, False)
V_SCHRAUD = True     # False: all exp on ACT
V_COLTILE = True     # False: ZW strips all at partition 0 (serial)
FILL0 = 0            # HAM filler matmuls before zw batch 0 (kernel is
FILL1 = 0            # PE-bound: fillers cost more than the warmth buys)

PACKQ_LAYOUT = [("xq0", NQ), ("xq1", NQ), ("wq0", DOUT), ("wq1", DOUT)]
PACKK_LAYOUT = [("wk0", DOUT), ("wk1", DOUT), ("uw", 9 * NKT), ("ef", 9)]
PACKQW = sum(w for _, w in PACKQ_LAYOUT)
PACKKW = sum(w for _, w in PACKK_LAYOUT)

_CACHE = {}


def _build_nc(legalize=True):
    nc = bass.Bass()
    # bf16 inputs in two packed tensors (xT separate so K-proj gets its
    # completion semaphore as early as possible)
    d_xp = [nc.declare_dram_parameter(f"pack_x{h}", [128, 2048], BF16,
                                      isOutput=False) for h in range(2)]
    d_pq = nc.declare_dram_parameter("pack_q", [128, PACKQW], BF16,
                                     isOutput=False)
    d_pk = nc.declare_dram_parameter("pack_k", [128, PACKKW], BF16,
                                     isOutput=False)
    d_pf = nc.declare_dram_parameter("pack_f32", [128, 5], F32, isOutput=False)
    d_out = nc.declare_dram_parameter("out", [NQ, 1], F32, isOutput=True)

    with tile.TileContext(nc) as tc:
        with ExitStack() as ctx:
            _body(ctx, tc, d_xp, d_pq, d_pk, d_pf, d_out)
    if legalize:
        _legalize_waits(nc)
    return nc


def _legalize_waits(nc):
    """walrus accepts a single sync wait per lowered instruction; split any
    extra waits onto injected same-engine NoOps placed just before."""
    cnt = 0
    skip = ("InstEventSemaphore", "InstNoOp", "InstISA")
    for f in nc.m.functions:
        for bb in f.blocks:
            out = []
            for ins in bb.instructions:
                si = getattr(ins, "sync_info", None)
                waits = list(si.on_wait) if (si is not None and si.on_wait) else []
                if len(waits) >= 2 and type(ins).__name__ not in skip:
                    for w in waits[:-1]:
                        nop = mybir.InstEventSemaphore(
                            name=f"wsplit_{cnt}", ins=[], outs=[])
                        cnt += 1
                        nop.engine = ins.engine
                        nop.sync_info = mybir.SyncInfo(on_wait=[w], on_update=[])
                        out.append(nop)
                    ins.sync_info = mybir.SyncInfo(
                        on_wait=[waits[-1]], on_update=list(si.on_update or []))
                out.append(ins)
            bb.instructions[:] = out
    return nc


def _body(ctx, tc, d_xp, d_pq, d_pk, d_pf, d_out):
    nc = tc.nc
    AF = mybir.ActivationFunctionType
    OP = mybir.AluOpType

    const_pool = ctx.enter_context(tc.tile_pool(name="const", bufs=1))
    persist = ctx.enter_context(tc.tile_pool(name="persist", bufs=1))

    # ---- input DMAs (xT split by key-half: K-proj half 0 starts after
    # only the first 0.5MB lands) ----
    pq_t = const_pool.tile([128, PACKQW], BF16)
    nc.sync.dma_start(pq_t[:], d_pq[:])
    pf = const_pool.tile([128, 5], F32)
    nc.sync.dma_start(pf[:], d_pf[:])
    pk_t = const_pool.tile([128, PACKKW], BF16)
    nc.sync.dma_start(pk_t[:], d_pk[:])
    xp = [const_pool.tile([128, 2048], BF16, name=f"xp{h}", tag=f"xp{h}")
          for h in range(2)]
    for h in range(2):
        nc.sync.dma_start(xp[h][:], d_xp[h][:])

    offq, o = {}, 0
    for nm, w in PACKQ_LAYOUT:
        offq[nm] = o
        o += w
    offk, o = {}, 0
    for nm, w in PACKK_LAYOUT:
        offk[nm] = o
        o += w
    xqT = [pq_t[:, offq[f"xq{c}"]:offq[f"xq{c}"] + NQ] for c in range(2)]
    wq_bf = [pq_t[:, offq[f"wq{c}"]:offq[f"wq{c}"] + DOUT] for c in range(2)]
    wk_bf = [pk_t[:, offk[f"wk{c}"]:offk[f"wk{c}"] + DOUT] for c in range(2)]
    uw = pk_t[:, offk["uw"]:offk["uw"] + 9 * NKT]
    efold = pk_t[:, offk["ef"]:offk["ef"] + 9]
    bq_col = pf[:, 0:4]
    bo_rep = pf[:, 4:5]
    xTh = [[xp[h][:, c * 1024:(c + 1) * 1024] for h in range(2)]
           for c in range(2)]

    # zeros row for the zw bank-zeroing matmul
    zrow = persist.tile([1, 128], BF16)
    nc.vector.memset(zrow[:], 0.0)

    # ACT warm-up: trigger the exp table load early (overlaps input DMA)
    actw = persist.tile([1, 1], F32)
    nc.scalar.activation(actw[:], zrow[0:1, 0:1], AF.Exp, bias=0.0, scale=1.0)

    # ---- persistent SBUF ----
    QT = [persist.tile([128, NQ], BF16, name=f"QT{d}", tag=f"QT{d}")
          for d in range(4)]
    # KT[d][h]: [128, 1024] half h of K^T for head pair d
    KT = [[persist.tile([128, 1024], BF16, name=f"KT{d}_{h}", tag=f"KT{d}_{h}")
           for h in range(2)] for d in range(4)]
    # p[d][t]: exp'd scores, [128 keys, 1024 = 2kt x 2hh x 256q] bf16
    PP = [[persist.tile([128, 1024], BF16, name=f"p{d}_{t}", tag=f"p{d}_{t}")
           for t in range(8)] for d in range(4)]
    zw_sb = persist.tile([128, 4 * 512], BF16)
    res = persist.tile([128, 2], F32)

    stp = ctx.enter_context(tc.tile_pool(name="stp", bufs=3, space="PSUM"))
    zwp = ctx.enter_context(tc.tile_pool(name="zwp", bufs=1, space="PSUM"))
    ztp = ctx.enter_context(tc.tile_pool(name="ztp", bufs=1, space="PSUM"))

    # ---- Q projection: all 4 head-pairs in one stp tile (quarters of
    # two banks), stagings emitted together so neither engine straggles ----
    pq = stp.tile([128, 1024], F32, tag="st")
    for d in range(4):
        for c in range(2):
            nc.tensor.matmul(pq[:, d * NQ:(d + 1) * NQ],
                             wq_bf[c][:, d * 128:(d + 1) * 128],
                             xqT[c][:], start=(c == 0), stop=(c == 1))
    for d in range(4):
        if d % 2 == 0:
            nc.scalar.activation(QT[d][:], pq[:, d * NQ:(d + 1) * NQ],
                                 AF.Identity,
                                 bias=bq_col[:, d:d + 1], scale=1.0)
        else:
            nc.vector.tensor_scalar_add(QT[d][:], pq[:, d * NQ:(d + 1) * NQ],
                                        bq_col[:, d:d + 1])

    def kproj_mm(d, h):
        """half h: key chunks f = 2h, 2h+1 -> one [128,1024] stp tile"""
        pk = stp.tile([128, 1024], F32, tag="st")
        for i in range(2):
            f = 2 * h + i
            for c in range(2):
                nc.tensor.matmul(pk[:, i * 512:(i + 1) * 512],
                                 wk_bf[c][:, d * 128:(d + 1) * 128],
                                 xTh[c][h][:, i * 512:(i + 1) * 512],
                                 start=(c == 0), stop=(c == 1))
        return pk

    def kstage(d, h, pk):
        if KT_ON_ACT[h]:
            nc.scalar.activation(KT[d][h][:], pk[:], AF.Copy,
                                 bias=0.0, scale=1.0)
        else:
            nc.vector.tensor_copy(KT[d][h][:], pk[:])

    def kproj(d, h):
        kstage(d, h, kproj_mm(d, h))

    # K projection for d=0 (d+1 is projected during d)
    kproj(0, 0)
    kproj(0, 1)

    def scores_tile(d, t):
        """two key tiles (kt=2t, 2t+1), both heads -> st [128, 1024].
        Layout [h0kt0|h0kt1|h1kt0|h1kt1]: the concurrently-running
        head-pair matmuls (row groups 0/64) land in different banks --
        concurrent PE writes into one bank are a device fault."""
        st = stp.tile([128, 1024], F32, tag="st")
        for j in range(2):
            kt = 2 * t + j
            h, o = kt // 8, (kt % 8) * 128
            for hh in range(2):
                nc.tensor.matmul(
                    st[:, hh * 512 + j * NQ:hh * 512 + (j + 1) * NQ],
                    KT[d][h][hh * HD:(hh + 1) * HD, o:o + 128],
                    QT[d][hh * HD:(hh + 1) * HD, :])
        return st

    def exp_tile(d, t, st):
        p = PP[d][t]
        if EXP_ON_ACT[t] or not V_SCHRAUD:
            nc.scalar.activation(p[:], st[:], AF.Exp, bias=0.0, scale=SC_EXP)
        else:
            nc.vector.tensor_scalar(p[:].bitcast(I16), st[:],
                                    B_IMM, A_IMM, op0=OP.add, op1=OP.mult)

    def zw_batch(d, zw_d, b):
        """key tiles 4b..4b+3 -> 4 concurrent col-tiled strip matmuls"""
        for s in range(4):
            kt = 4 * b + s
            p = PP[d][kt // 2]
            j = kt % 2
            # moving = [h0 block j | h1 block j]: [128, 2, 256] AP
            pv = p[:].rearrange("p (h jq) -> p h jq", h=2)
            pv = pv[:, :, j * NQ:(j + 1) * NQ]
            so = 32 * s if V_COLTILE else 0
            nc.tensor.matmul(
                zw_d[so:so + 9, :],
                uw[:, kt * 9:kt * 9 + 9],
                pv,
                start=False, stop=(kt == NKT - 1),
                tile_position=(0, so), skip_group_check=True)

    # ---- main pipeline ----
    def zw_zero(d):
        """open the zw accumulator bank: zero it with a start=True matmul
        (also absorbs the bank WAR into a cheap PE op)"""
        zw_d = zwp.tile([128, 512], F32, tag="zw", name=f"zw{d}")
        nc.tensor.matmul(zw_d[:], zrow[:], wq_bf[0][0:1, 0:512],
                         start=True, stop=False, skip_group_check=True)
        return zw_d

    def zw_store(d, zw_d, on_act):
        if on_act:
            nc.scalar.activation(zw_sb[:, d * 512:(d + 1) * 512], zw_d[:],
                                 AF.Copy, bias=0.0, scale=1.0)
        else:
            nc.vector.tensor_copy(zw_sb[:, d * 512:(d + 1) * 512], zw_d[:])

    def zw_fill(zw_d, n=1):
        """HAM-warming filler: accumulate zeros into the live zw bank.
        Mathematically a no-op; keeps the PE busy across exp-wait stalls
        so the clock gate stays at 8/8."""
        for _ in range(n):
            nc.tensor.matmul(zw_d[:], zrow[:], wq_bf[0][0:1, 0:512],
                             start=False, stop=False, skip_group_check=True)

    zt = ztp.tile([128, 192], F32, tag="zt")
    zw = {}
    for d in range(4):
        def tile(t):
            exp_tile(d, t, scores_tile(d, t))
        tile(0)
        tile(1)
        tile(2)
        tile(3)
        zw[d] = zw_zero(d)
        zw_fill(zw[d], FILL0)
        zw_batch(d, zw[d], 0)
        if d < 3:
            kproj(d + 1, 0)
        tile(4)
        tile(5)
        if d < 3:
            kproj(d + 1, 1)
        zw_fill(zw[d], FILL1)
        zw_batch(d, zw[d], 1)
        tile(6)
        tile(7)
        if d > 0:
            for ch in range(4):
                dd = d - 1
                nc.tensor.matmul(
                    zt[:, (4 * dd + ch) * 9:(4 * dd + ch) * 9 + 9],
                    zw_sb[:, dd * 512 + ch * 128:dd * 512 + (ch + 1) * 128],
                    efold[:], skip_group_check=True)
        zw_fill(zw[d], FILL1)
        zw_batch(d, zw[d], 2)
        zw_fill(zw[d], FILL1)
        zw_batch(d, zw[d], 3)
        # odd d (incl. the final one) stores on ACT: its exp queue drains
        # one tile earlier than DVE's, so the last store isn't serialized
        # behind exp t7
        zw_store(d, zw[d], d % 2 == 1)
    for ch in range(4):
        nc.tensor.matmul(zt[:, (12 + ch) * 9:(12 + ch) * 9 + 9],
                         zw_sb[:, 3 * 512 + ch * 128:3 * 512 + (ch + 1) * 128],
                         efold[:], skip_group_check=True)

    # ---- final combine: out[q] = sum_h W[q,h]/Z[q,h] + bo ----
    # zt col = 18H + 9qc + r (H = head, qc = query half, r = 0:Z, 1+H:W)
    ld = ctx.enter_context(tc.tile_pool(name="ld", bufs=1))
    zr = ld.tile([128, 16], F32, tag="zr")
    nc.vector.reciprocal(zr[:], zt[:, 0:136:9])           # (H, qc) pairs
    w_ap = zt[:, 1:1 + 19 * 8].rearrange("p (h r) -> p h r", h=8)[:, :, 0:10:9]
    wz = ld.tile([128, 16], F32, tag="wz")                # layout [qc, H]
    nc.vector.tensor_mul(wz[:].rearrange("p (q h) -> p h q", q=2), w_ap,
                         zr[:].rearrange("p (h q) -> p h q", h=8))
    sm = ld.tile([128, 2], F32, tag="sm")
    nc.vector.reduce_sum(sm[:], wz[:].rearrange("p (q h) -> p q h", q=2),
                         axis=mybir.AxisListType.X)
    nc.vector.tensor_scalar_add(res[:], sm[:], bo_rep[:])
    nc.sync.dma_start(d_out.rearrange("(q p) o -> p (q o)", p=128), res[:])


def _host_prep(inputs):
    f32 = np.float32
    bf = ml_dtypes.bfloat16
    x = np.ascontiguousarray(inputs["x"], dtype=f32)
    Wo0 = inputs["Wo"][:, 0].astype(f32)
    wv_t = (inputs["Wv"].astype(f32) * Wo0[None, :]).reshape(CIN, H, HD).sum(-1)
    bv_t = (inputs["bv"].astype(f32) * Wo0).reshape(H, HD).sum(-1)
    # motion gate (host: O(N*small) input prep)
    mf = np.concatenate([inputs["rel_vel"], inputs["rel_angle"]], 1).astype(f32)
    z = np.maximum(mf @ inputs["Wmg1"].astype(f32) + inputs["bmg1"], 0.0)
    z = z @ inputs["Wmg2"].astype(f32) + inputs["bmg2"]
    mg = 1.0 / (1.0 + np.exp(-z))                      # (N, 1)
    U = mg * (x @ wv_t + bv_t)                         # (N, 8) gated
    uw_full = np.concatenate([np.ones((N, 1), f32), U], 1)   # (N, 9)
    uw_pack = uw_full.reshape(NKT, 128, 9).transpose(1, 0, 2).reshape(128, -1)
    E = np.zeros((128, 9), f32)
    for s in range(4):
        E[32 * s:32 * s + 9, :] = np.eye(9, dtype=f32)
    xt_bf = np.ascontiguousarray(x.T).astype(bf)
    wq = inputs["Wq"].astype(f32)
    wk = inputs["Wk"].astype(f32)
    pfv = np.concatenate(
        [inputs["bq"].astype(f32).reshape(4, 128).T,
         np.full((128, 1), inputs["bo"][0], f32)], axis=1)
    common = dict(
        xt_bf_local=xt_bf,
        pack_f32=np.ascontiguousarray(pfv),
    )
    for h in range(2):
        common[f"pack_x{h}"] = np.ascontiguousarray(
            np.concatenate([xt_bf[0:128, h * 1024:(h + 1) * 1024].astype(f32),
                            xt_bf[128:256, h * 1024:(h + 1) * 1024].astype(f32)],
                           axis=1)).astype(bf)
    common["pack_k"] = np.ascontiguousarray(np.concatenate(
        [wk[0:128], wk[128:256], uw_pack, E], axis=1)).astype(bf)
    common["_parts"] = dict(wq0=wq[0:128], wq1=wq[128:256])
    return common


def kernel(**inputs):
    if "nc" not in _CACHE:
        _CACHE["nc"] = _build_nc()
    nc = _CACHE["nc"]
    common = _host_prep(inputs)
    xt = common.pop("xt_bf_local")
    parts = common.pop("_parts")
    bf = ml_dtypes.bfloat16
    in_maps = []
    for i in range(NCORES):
        xq = xt[:, i * NQ:(i + 1) * NQ].astype(np.float32)
        parts["xq0"], parts["xq1"] = xq[0:128], xq[128:256]
        packed = np.concatenate([np.asarray(parts[nm], dtype=np.float32)
                                 for nm, _ in PACKQ_LAYOUT], axis=1)
        in_maps.append(dict(common,
                            pack_q=np.ascontiguousarray(packed).astype(bf)))
    res = run_bass_kernel_spmd(nc, in_maps, core_ids=list(range(NCORES)),
                               **_CACHE.get("run_kwargs", {}))
    _CACHE["last_results"] = res
    out = np.concatenate([np.asarray(res.results[i]["out"])[:, 0]
                          for i in range(NCORES)])
    return out.astype(np.float32)


# revision 60
# speedup vs baseline: 1.0874x; 1.0874x over previous
"""Trainium2 Bass kernel for a multi-head cross-attention module.

Math (validated vs reference to 5.4e-7 in f32):
  Q = x@Wq+bq, K = x@Wk          (N=2048, 8 heads, head_dim=64)
  scores[q,k,h] = <Q[q,h,:], K[k,h,:]>/8
    - spatial bias sb(q): per-query shift along k -> softmax no-op, dropped
    - K bias bk: <Q[q,h],bk[h]> is per-(q,h) shift along k -> softmax
      no-op, dropped (exact)
  A = softmax_k(scores); out[q] = sum_{k,h} A[q,k,h]*U[k,h]/Z[q,h] + bo
  where U[k,h] = mg[k]*(x[k]@Wv_tilde[:,h]+bv_tilde[h]) folds the V
  projection, motion gate and output projection (host-prepped: the
  gate MLP + U are O(N*small), 0.4% of total FLOPs; all O(N*d^2)
  projections and the O(N^2*H) attention run on device).

Sharding: queries split 256/core across 8 cores; K/U replicated.

Per-core dataflow (d = head-pair 0..3 pipelined):
  K-proj (PE, bf16) -> KT staging (ACT/DVE split) ->
  scores S^T[k,q] per key-tile, head pair concurrent on PE row-groups
  (64-row contraction at base partitions 0/64) ->
  exp: even tiles ACT Exp(scale=1/8); odd tiles DVE "Schraudolph"
  (one tensor_scalar producing the bf16 BITS of exp via int16 convert +
  bitcast; end-to-end rel err contribution ~1e-3) ->
  Z/W matmul against [1|U] with 4x PE column-tiling: key-tile kt goes to
  partition strip 32*(kt%4), 4 concurrent streams, one zeroing matmul
  opens the bank -> strips folded by a [128,9] 4-stacked-identity
  matmul (E) which also transposes for the final combine.

Walrus 1-wait constraint handled by _legalize_waits; steady-state the
schedule needs <=1 wait per instruction (vector clocks elide repeats).
"""

import numpy as np
import ml_dtypes
from contextlib import ExitStack

import concourse.bass as bass
import concourse.mybir as mybir
import concourse.tile as tile
from concourse.bass_utils import run_bass_kernel_spmd

N = 2048
CIN = 256
DOUT = 512
H = 8
HD = 64
NCORES = 8
NQ = N // NCORES        # 256 queries per core
NKT = N // 128          # 16 key tiles
F32 = mybir.dt.float32
BF16 = mybir.dt.bfloat16
I16 = mybir.dt.int16
F8 = mybir.dt.float8e4

# Schraudolph: bf16bits(exp(s/8)) ~= int16((s + B) * A)
SC_EXP = 0.125
A_IMM = SC_EXP * 128.0 / float(np.log(2.0))
B_IMM = 16249.0 / A_IMM                    # (127*128 - 7)/A

# engine split per d-iteration (tunable): exp tiles t=0..7, KT chunks f=0..3
EXP_ON_ACT = (True, False, True, False, True, False, True, False)
KT_ON_ACT = (True, False)
V_SCHRAUD = True     # False: all exp on ACT
V_COLTILE = True     # False: ZW strips all at partition 0 (serial)
FILL0 = 0            # HAM filler matmuls before zw batch 0 (kernel is
FILL1 = 0            # PE-bound: fillers cost more than the warmth buys)

PACKQ_LAYOUT = [("xq0", NQ), ("xq1", NQ), ("wq0", DOUT), ("wq1", DOUT)]
PACKK_LAYOUT = [("wk0", DOUT), ("wk1", DOUT), ("uw", 9 * NKT), ("ef", 9)]
PACKQW = sum(w for _, w in PACKQ_LAYOUT)
PACKKW = sum(w for _, w in PACKK_LAYOUT)

_CACHE = {}


def _build_nc(legalize=True):
    nc = bass.Bass()
    # bf16 inputs in two packed tensors (xT separate so K-proj gets its
    # completion semaphore as early as possible)
    d_xp = [nc.declare_dram_parameter(f"pack_x{h}", [128, 2048], BF16,
                                      isOutput=False) for h in range(2)]
    d_pq = nc.declare_dram_parameter("pack_q", [128, PACKQW], BF16,
                                     isOutput=False)
    d_pk = nc.declare_dram_parameter("pack_k", [128, PACKKW], BF16,
                                     isOutput=False)
    d_pf = nc.declare_dram_parameter("pack_f32", [128, 5], F32, isOutput=False)
    d_out = nc.declare_dram_parameter("out", [NQ, 1], F32, isOutput=True)

    with tile.TileContext(nc) as tc:
        with ExitStack() as ctx:
            _body(ctx, tc, d_xp, d_pq, d_pk, d_pf, d_out)
    if legalize:
        _legalize_waits(nc)
    return nc


def _legalize_waits(nc):
    """walrus accepts a single sync wait per lowered instruction; split any
    extra waits onto injected same-engine NoOps placed just before."""
    cnt = 0
    skip = ("InstEventSemaphore", "InstNoOp", "InstISA")
    for f in nc.m.functions:
        for bb in f.blocks:
            out = []
            for ins in bb.instructions:
                si = getattr(ins, "sync_info", None)
                waits = list(si.on_wait) if (si is not None and si.on_wait) else []
                if len(waits) >= 2 and type(ins).__name__ not in skip:
                    for w in waits[:-1]:
                        nop = mybir.InstEventSemaphore(
                            name=f"wsplit_{cnt}", ins=[], outs=[])
                        cnt += 1
                        nop.engine = ins.engine
                        nop.sync_info = mybir.SyncInfo(on_wait=[w], on_update=[])
                        out.append(nop)
                    ins.sync_info = mybir.SyncInfo(
                        on_wait=[waits[-1]], on_update=list(si.on_update or []))
                out.append(ins)
            bb.instructions[:] = out
    return nc


def _body(ctx, tc, d_xp, d_pq, d_pk, d_pf, d_out):
    nc = tc.nc
    AF = mybir.ActivationFunctionType
    OP = mybir.AluOpType

    const_pool = ctx.enter_context(tc.tile_pool(name="const", bufs=1))
    persist = ctx.enter_context(tc.tile_pool(name="persist", bufs=1))

    # ---- input DMAs (xT split by key-half: K-proj half 0 starts after
    # only the first 0.5MB lands) ----
    pq_t = const_pool.tile([128, PACKQW], BF16)
    nc.sync.dma_start(pq_t[:], d_pq[:])
    pf = const_pool.tile([128, 5], F32)
    nc.sync.dma_start(pf[:], d_pf[:])
    pk_t = const_pool.tile([128, PACKKW], BF16)
    nc.sync.dma_start(pk_t[:], d_pk[:])
    xp = [const_pool.tile([128, 2048], BF16, name=f"xp{h}", tag=f"xp{h}")
          for h in range(2)]
    for h in range(2):
        nc.sync.dma_start(xp[h][:], d_xp[h][:])

    offq, o = {}, 0
    for nm, w in PACKQ_LAYOUT:
        offq[nm] = o
        o += w
    offk, o = {}, 0
    for nm, w in PACKK_LAYOUT:
        offk[nm] = o
        o += w
    xqT = [pq_t[:, offq[f"xq{c}"]:offq[f"xq{c}"] + NQ] for c in range(2)]
    wq_bf = [pq_t[:, offq[f"wq{c}"]:offq[f"wq{c}"] + DOUT] for c in range(2)]
    wk_bf = [pk_t[:, offk[f"wk{c}"]:offk[f"wk{c}"] + DOUT] for c in range(2)]
    uw = pk_t[:, offk["uw"]:offk["uw"] + 9 * NKT]
    efold = pk_t[:, offk["ef"]:offk["ef"] + 9]
    bq_col = pf[:, 0:4]
    bo_rep = pf[:, 4:5]
    xTh = [[xp[h][:, c * 1024:(c + 1) * 1024] for h in range(2)]
           for c in range(2)]

    # zeros row for the zw bank-zeroing matmul
    zrow = persist.tile([1, 128], BF16)
    nc.vector.memset(zrow[:], 0.0)

    # ACT warm-up: trigger the exp table load early (overlaps input DMA)
    actw = persist.tile([1, 1], F32)
    nc.scalar.activation(actw[:], zrow[0:1, 0:1], AF.Exp, bias=0.0, scale=1.0)

    # ---- persistent SBUF ----
    QT = [persist.tile([128, NQ], BF16, name=f"QT{d}", tag=f"QT{d}")
          for d in range(4)]
    # KT[d][h]: [128, 1024] half h of K^T for head pair d
    KT = [[persist.tile([128, 1024], BF16, name=f"KT{d}_{h}", tag=f"KT{d}_{h}")
           for h in range(2)] for d in range(4)]
    # p[d][t]: exp'd scores, [128 keys, 1024 = 2kt x 2hh x 256q] bf16
    PP = [[persist.tile([128, 1024], BF16, name=f"p{d}_{t}", tag=f"p{d}_{t}")
           for t in range(8)] for d in range(4)]
    zw_sb = persist.tile([128, 4 * 512], BF16)
    res = persist.tile([128, 2], F32)

    stp = ctx.enter_context(tc.tile_pool(name="stp", bufs=3, space="PSUM"))
    zwp = ctx.enter_context(tc.tile_pool(name="zwp", bufs=1, space="PSUM"))
    ztp = ctx.enter_context(tc.tile_pool(name="ztp", bufs=1, space="PSUM"))

    # ---- Q projection: all 4 head-pairs in one stp tile (quarters of
    # two banks), stagings emitted together so neither engine straggles ----
    pq = stp.tile([128, 1024], F32, tag="st")
    for d in range(4):
        for c in range(2):
            nc.tensor.matmul(pq[:, d * NQ:(d + 1) * NQ],
                             wq_bf[c][:, d * 128:(d + 1) * 128],
                             xqT[c][:], start=(c == 0), stop=(c == 1))
    for d in range(4):
        if d % 2 == 0:
            nc.scalar.activation(QT[d][:], pq[:, d * NQ:(d + 1) * NQ],
                                 AF.Identity,
                                 bias=bq_col[:, d:d + 1], scale=1.0)
        else:
            nc.vector.tensor_scalar_add(QT[d][:], pq[:, d * NQ:(d + 1) * NQ],
                                        bq_col[:, d:d + 1])

    def kproj_mm(d, h):
        """half h: key chunks f = 2h, 2h+1 -> one [128,1024] stp tile"""
        pk = stp.tile([128, 1024], F32, tag="st")
        for i in range(2):
            f = 2 * h + i
            for c in range(2):
                nc.tensor.matmul(pk[:, i * 512:(i + 1) * 512],
                                 wk_bf[c][:, d * 128:(d + 1) * 128],
                                 xTh[c][h][:, i * 512:(i + 1) * 512],
                                 start=(c == 0), stop=(c == 1))
        return pk

    def kstage(d, h, pk):
        if KT_ON_ACT[h]:
            nc.scalar.activation(KT[d][h][:], pk[:], AF.Copy,
                                 bias=0.0, scale=1.0)
        else:
            nc.vector.tensor_copy(KT[d][h][:], pk[:])

    def kproj(d, h):
        kstage(d, h, kproj_mm(d, h))

    # K projection for d=0 (d+1 is projected during d)
    kproj(0, 0)
    kproj(0, 1)

    def scores_tile(d, t):
        """two key tiles (kt=2t, 2t+1), both heads -> st [128, 1024].
        Layout [h0kt0|h0kt1|h1kt0|h1kt1]: the concurrently-running
        head-pair matmuls (row groups 0/64) land in different banks --
        concurrent PE writes into one bank are a device fault."""
        st = stp.tile([128, 1024], F32, tag="st")
        for j in range(2):
            kt = 2 * t + j
            h, o = kt // 8, (kt % 8) * 128
            for hh in range(2):
                nc.tensor.matmul(
                    st[:, hh * 512 + j * NQ:hh * 512 + (j + 1) * NQ],
                    KT[d][h][hh * HD:(hh + 1) * HD, o:o + 128],
                    QT[d][hh * HD:(hh + 1) * HD, :])
        return st

    def exp_tile(d, t, st):
        p = PP[d][t]
        if EXP_ON_ACT[t] or not V_SCHRAUD:
            nc.scalar.activation(p[:], st[:], AF.Exp, bias=0.0, scale=SC_EXP)
        else:
            nc.vector.tensor_scalar(p[:].bitcast(I16), st[:],
                                    B_IMM, A_IMM, op0=OP.add, op1=OP.mult)

    def zw_batch(d, zw_d, b):
        """key tiles 4b..4b+3 -> 4 concurrent col-tiled strip matmuls"""
        for s in range(4):
            kt = 4 * b + s
            p = PP[d][kt // 2]
            j = kt % 2
            # moving = [h0 block j | h1 block j]: [128, 2, 256] AP
            pv = p[:].rearrange("p (h jq) -> p h jq", h=2)
            pv = pv[:, :, j * NQ:(j + 1) * NQ]
            so = 32 * s if V_COLTILE else 0
            nc.tensor.matmul(
                zw_d[so:so + 9, :],
                uw[:, kt * 9:kt * 9 + 9],
                pv,
                start=False, stop=(kt == NKT - 1),
                tile_position=(0, so), skip_group_check=True)

    # ---- main pipeline ----
    def zw_zero(d):
        """open the zw accumulator bank: zero it with a start=True matmul
        (also absorbs the bank WAR into a cheap PE op)"""
        zw_d = zwp.tile([128, 512], F32, tag="zw", name=f"zw{d}")
        nc.tensor.matmul(zw_d[:], zrow[:], wq_bf[0][0:1, 0:512],
                         start=True, stop=False, skip_group_check=True)
        return zw_d

    def zw_store(d, zw_d, on_act):
        if on_act:
            nc.scalar.activation(zw_sb[:, d * 512:(d + 1) * 512], zw_d[:],
                                 AF.Copy, bias=0.0, scale=1.0)
        else:
            nc.vector.tensor_copy(zw_sb[:, d * 512:(d + 1) * 512], zw_d[:])

    def zw_fill(zw_d, n=1):
        """HAM-warming filler: accumulate zeros into the live zw bank.
        Mathematically a no-op; keeps the PE busy across exp-wait stalls
        so the clock gate stays at 8/8."""
        for _ in range(n):
            nc.tensor.matmul(zw_d[:], zrow[:], wq_bf[0][0:1, 0:512],
                             start=False, stop=False, skip_group_check=True)

    zt = ztp.tile([128, 192], F32, tag="zt")
    zw = {}
    for d in range(4):
        def tile(t):
            exp_tile(d, t, scores_tile(d, t))
        tile(0)
        tile(1)
        tile(2)
        tile(3)
        zw[d] = zw_zero(d)
        zw_fill(zw[d], FILL0)
        zw_batch(d, zw[d], 0)
        if d < 3:
            kproj(d + 1, 0)
        tile(4)
        tile(5)
        zw_fill(zw[d], FILL1)
        zw_batch(d, zw[d], 1)
        if d < 3:
            kproj(d + 1, 1)
        tile(6)
        tile(7)
        zw_fill(zw[d], FILL1)
        zw_batch(d, zw[d], 2)
        if d > 0:
            for ch in range(4):
                dd = d - 1
                nc.tensor.matmul(
                    zt[:, (4 * dd + ch) * 9:(4 * dd + ch) * 9 + 9],
                    zw_sb[:, dd * 512 + ch * 128:dd * 512 + (ch + 1) * 128],
                    efold[:], skip_group_check=True)
        zw_fill(zw[d], FILL1)
        zw_batch(d, zw[d], 3)
        # odd d (incl. the final one) stores on ACT: its exp queue drains
        # one tile earlier than DVE's, so the last store isn't serialized
        # behind exp t7
        zw_store(d, zw[d], d % 2 == 1)
    for ch in range(4):
        nc.tensor.matmul(zt[:, (12 + ch) * 9:(12 + ch) * 9 + 9],
                         zw_sb[:, 3 * 512 + ch * 128:3 * 512 + (ch + 1) * 128],
                         efold[:], skip_group_check=True)

    # ---- final combine: out[q] = sum_h W[q,h]/Z[q,h] + bo ----
    # zt col = 18H + 9qc + r (H = head, qc = query half, r = 0:Z, 1+H:W)
    ld = ctx.enter_context(tc.tile_pool(name="ld", bufs=1))
    zr = ld.tile([128, 16], F32, tag="zr")
    nc.vector.reciprocal(zr[:], zt[:, 0:136:9])           # (H, qc) pairs
    w_ap = zt[:, 1:1 + 19 * 8].rearrange("p (h r) -> p h r", h=8)[:, :, 0:10:9]
    wz = ld.tile([128, 16], F32, tag="wz")                # layout [qc, H]
    nc.vector.tensor_mul(wz[:].rearrange("p (q h) -> p h q", q=2), w_ap,
                         zr[:].rearrange("p (h q) -> p h q", h=8))
    sm = ld.tile([128, 2], F32, tag="sm")
    nc.vector.reduce_sum(sm[:], wz[:].rearrange("p (q h) -> p q h", q=2),
                         axis=mybir.AxisListType.X)
    nc.vector.tensor_scalar_add(res[:], sm[:], bo_rep[:])
    nc.sync.dma_start(d_out.rearrange("(q p) o -> p (q o)", p=128), res[:])


def _host_prep(inputs):
    f32 = np.float32
    bf = ml_dtypes.bfloat16
    x = np.ascontiguousarray(inputs["x"], dtype=f32)
    Wo0 = inputs["Wo"][:, 0].astype(f32)
    wv_t = (inputs["Wv"].astype(f32) * Wo0[None, :]).reshape(CIN, H, HD).sum(-1)
    bv_t = (inputs["bv"].astype(f32) * Wo0).reshape(H, HD).sum(-1)
    # motion gate (host: O(N*small) input prep)
    mf = np.concatenate([inputs["rel_vel"], inputs["rel_angle"]], 1).astype(f32)
    z = np.maximum(mf @ inputs["Wmg1"].astype(f32) + inputs["bmg1"], 0.0)
    z = z @ inputs["Wmg2"].astype(f32) + inputs["bmg2"]
    mg = 1.0 / (1.0 + np.exp(-z))                      # (N, 1)
    U = mg * (x @ wv_t + bv_t)                         # (N, 8) gated
    uw_full = np.concatenate([np.ones((N, 1), f32), U], 1)   # (N, 9)
    uw_pack = uw_full.reshape(NKT, 128, 9).transpose(1, 0, 2).reshape(128, -1)
    E = np.zeros((128, 9), f32)
    for s in range(4):
        E[32 * s:32 * s + 9, :] = np.eye(9, dtype=f32)
    xt_bf = np.ascontiguousarray(x.T).astype(bf)
    wq = inputs["Wq"].astype(f32)
    wk = inputs["Wk"].astype(f32)
    pfv = np.concatenate(
        [inputs["bq"].astype(f32).reshape(4, 128).T,
         np.full((128, 1), inputs["bo"][0], f32)], axis=1)
    common = dict(
        xt_bf_local=xt_bf,
        pack_f32=np.ascontiguousarray(pfv),
    )
    for h in range(2):
        common[f"pack_x{h}"] = np.ascontiguousarray(
            np.concatenate([xt_bf[0:128, h * 1024:(h + 1) * 1024].astype(f32),
                            xt_bf[128:256, h * 1024:(h + 1) * 1024].astype(f32)],
                           axis=1)).astype(bf)
    common["pack_k"] = np.ascontiguousarray(np.concatenate(
        [wk[0:128], wk[128:256], uw_pack, E], axis=1)).astype(bf)
    common["_parts"] = dict(wq0=wq[0:128], wq1=wq[128:256])
    return common


def kernel(**inputs):
    if "nc" not in _CACHE:
        _CACHE["nc"] = _build_nc()
    nc = _CACHE["nc"]
    common = _host_prep(inputs)
    xt = common.pop("xt_bf_local")
    parts = common.pop("_parts")
    bf = ml_dtypes.bfloat16
    in_maps = []
    for i in range(NCORES):
        xq = xt[:, i * NQ:(i + 1) * NQ].astype(np.float32)
        parts["xq0"], parts["xq1"] = xq[0:128], xq[128:256]
        packed = np.concatenate([np.asarray(parts[nm], dtype=np.float32)
                                 for nm, _ in PACKQ_LAYOUT], axis=1)
        in_maps.append(dict(common,
                            pack_q=np.ascontiguousarray(packed).astype(bf)))
    res = run_bass_kernel_spmd(nc, in_maps, core_ids=list(range(NCORES)),
                               **_CACHE.get("run_kwargs", {}))
    _CACHE["last_results"] = res
    out = np.concatenate([np.asarray(res.results[i]["out"])[:, 0]
                          for i in range(NCORES)])
    return out.astype(np.float32)
